# revision 31
# baseline (speedup 1.0000x reference)
"""Trainium2 Bass kernel for a CLIP encoder layer (B=32, S=257, E=1024, H=16, I=4096).

Strategy: data-parallel over batch across 8 NeuronCores (4 batch elements per
core), no collectives.  Per-core compute is feature-major ([E, tokens]).

Quantization: Q/K/V/O/fc1 matmuls run in fp8(e4m3) with DoubleRow perf mode
(2 k-subtiles contracted per PE pass, 2x bf16 throughput); fc2 stays bf16
(fp8 there would blow the error budget).  Per-tensor scales fold into the
weights host-side; dequant scales fold into existing PSUM->SBUF ops (exp /
gelu / output adds), so no extra passes.

  - Activations live in 3D tiles: xT3/ht3 [128, KC, NT] f32 (LN source +
    residual), x3/xln2_3/ctx3 [128, KC, SP] fp8 whose k-pair slices feed
    DoubleRow matmuls directly.
  - LayerNorm: column sums via f32r ones-matmul straight on the f32 source,
    sum-of-squares via bf16 ones-matmul on an ACT-squared copy (both stats
    share one PSUM bank via column tile groups); mean/rstd broadcast via PE;
    the normalize runs as fused 4-k-wide DVE ops with stride-0 broadcast.
  - Q/K/O/fc1: weight-stationary DoubleRow matmuls, k-pair-outer loops so
    each LDWEIGHTS serves 2-4 matmuls.
  - V: activation-stationary DoubleRow -> token-major [tok, H, 128] tiles
    with a 1/S_CTX column so softmax denominators fall out of the ctx matmul
    (and ctx lands pre-scaled for fp8).
  - Attention: scores transposed, softmax over partition dim via ctx matmul,
    exp on ACT from PSUM with qk dequant folded into its scale.  All
    attention PSUM tiles are single-bank [128, 512] with deep pool rotation
    (pp2 x3 + psp x5) so score/exp/ctx chains pipeline across batches.
  - PE warmth: V interleaves with LN1 per batch; spinner matmuls fill
    DVE/ACT-heavy windows so the HAM clock gate stays at 8/8.
"""

import numpy as np
import ml_dtypes

B, S, E, H, D, II = 32, 257, 1024, 16, 64, 4096
N_CORES = 8
B_LOC = B // N_CORES          # 4
NT = B_LOC * S                # 1028
KC = E // 128                 # 8
MC_E = E // 128               # 8
MC_I = II // 128              # 32
EPS = 1e-5
SP = 272                      # S padded so fp8 k-slice stride is 16B-aligned

S_X = 16.0                    # LN output fp8 scale
S_CTX = 128.0                 # ctx fp8 scale

# j-chunks of one batch element's 257 keys
JC = [(0, 128), (128, 128), (256, 1)]

TRACE = False
LAST_EXEC_NS = None

_cache = {}


def _build(with_mask, with_vbias, with_qkbias, with_obias,
           dq_v, dq_qk, dq_o, dq_f1):
    import concourse.tile as tile
    from concourse import bacc, mybir
    from contextlib import ExitStack

    F32 = mybir.dt.float32
    F32R = mybir.dt.float32r
    BF16 = mybir.dt.bfloat16
    FP8 = mybir.dt.float8e4
    AF = mybir.ActivationFunctionType
    ALU = mybir.AluOpType
    DR = mybir.MatmulPerfMode.DoubleRow

    nc = bacc.Bacc("TRN2", target_bir_lowering=False, debug=False,
                   enable_asserts=False, num_devices=N_CORES)

    xT_d = nc.dram_tensor("xT", [E, NT], F32, kind="ExternalInput")
    xTb_d = nc.dram_tensor("xTb", [E, NT], BF16, kind="ExternalInput")
    qw_d = nc.dram_tensor("qw", [MC_E, 128, KC, 128], FP8, kind="ExternalInput")
    kw_d = nc.dram_tensor("kw", [MC_E, 128, KC, 128], FP8, kind="ExternalInput")
    vw_d = nc.dram_tensor("vw", [KC // 2, 128, 2, E], FP8, kind="ExternalInput")
    ow_d = nc.dram_tensor("ow", [MC_E, 128, KC, 128], FP8, kind="ExternalInput")
    f1w_d = nc.dram_tensor("f1w", [MC_I, 128, KC, 128], FP8, kind="ExternalInput")
    f2w_d = nc.dram_tensor("f2w", [MC_E, 128, MC_I, 128], BF16, kind="ExternalInput")
    qb_d = nc.dram_tensor("qb", [128, MC_E], F32, kind="ExternalInput")
    kb_d = nc.dram_tensor("kb", [128, MC_E], F32, kind="ExternalInput")
    vb_d = nc.dram_tensor("vb", [1, E], F32, kind="ExternalInput")
    ob_d = nc.dram_tensor("ob", [128, MC_E], F32, kind="ExternalInput")
    f1b_d = nc.dram_tensor("f1b", [128, MC_I], F32, kind="ExternalInput")
    f2b_d = nc.dram_tensor("f2b", [128, MC_E], F32, kind="ExternalInput")
    mskT_d = None
    if with_mask:
        mskT_d = nc.dram_tensor("mskT", [B_LOC, S, S], F32, kind="ExternalInput")
    outT_d = nc.dram_tensor("outT", [E, NT], F32, kind="ExternalOutput")

    with tile.TileContext(nc) as tc, ExitStack() as top:
        consts = top.enter_context(tc.tile_pool(name="consts", bufs=1))

        ones_col = consts.tile([128, 1], BF16)
        nc.vector.memset(ones_col[:], 1.0)
        ones_row = consts.tile([1, 128], BF16)
        nc.vector.memset(ones_row[:], 1.0)
        eps_t = consts.tile([1, 1], F32)
        nc.vector.memset(eps_t[:], EPS)
        spin_x = consts.tile([128, 64], BF16)
        nc.vector.memset(spin_x[:], 0.0)
        qb_sb = consts.tile([128, MC_E], F32)
        nc.sync.dma_start(out=qb_sb[:], in_=qb_d[:])
        kb_sb = consts.tile([128, MC_E], F32)
        nc.sync.dma_start(out=kb_sb[:], in_=kb_d[:])
        ob_sb = consts.tile([128, MC_E], F32)
        nc.sync.dma_start(out=ob_sb[:], in_=ob_d[:])
        f2b_sb = consts.tile([128, MC_E], F32)
        nc.sync.dma_start(out=f2b_sb[:], in_=f2b_d[:])
        f1b_sb = consts.tile([128, MC_I], F32)
        nc.sync.dma_start(out=f1b_sb[:], in_=f1b_d[:])
        vb_sb = None
        if with_vbias:
            vb_sb = consts.tile([128, E], F32)
            nc.sync.dma_start(out=vb_sb[:],
                              in_=vb_d[0:1, :].to_broadcast((128, E)))



        def emit_spin(spin_t, n):
            """Dependency-free PE matmuls to keep the HAM clock gate warm
            through DVE/ACT-heavy windows."""
            for _ in range(n):
                nc.tensor.matmul(spin_t[0:1, 0:64], ones_col[:], spin_x[:],
                                 start=True, stop=True)

        def emit_ln(ph, src3, out3, sfx, apply3=None, spin_t=None, spin=0,
                    after_b=None):
            """Per-batch column LayerNorm over the feature (partition) dim.
            src3: [128, KC, NT] bf16 tile (stats input).  apply3: optional
            higher-precision source for the normalize (defaults to src3).
            Writes (x - mu) * rstd * S_X into out3[b][:, k, 0:S] (fp8); LN
            scale/bias are folded into the downstream weights host-side.
            after_b(b) emits follow-on work (e.g. V matmuls) interleaved."""
            if apply3 is None:
                apply3 = src3
            tmp_dt = BF16 if apply3 is src3 else F32
            lntmp = ph.enter_context(tc.tile_pool(name=f"lntmp{sfx}", bufs=2))
            sq_p = ph.enter_context(tc.tile_pool(name=f"sqp{sfx}", bufs=2))
            rows = ph.enter_context(tc.tile_pool(name=f"rows{sfx}", bufs=8))
            pstat = ph.enter_context(
                tc.tile_pool(name=f"pstat{sfx}", bufs=1, space="PSUM"))
            pbc = ph.enter_context(
                tc.tile_pool(name=f"pbc{sfx}", bufs=2, space="PSUM"))
            for b in range(B_LOC):
                bs = slice(b * S, (b + 1) * S)
                # squares on DVE (ACT is the scarcer engine here)
                sqb = sq_p.tile([128, KC, S], BF16, name="sqb", tag="sqb")
                nc.vector.tensor_mul(out=sqb[:], in0=src3[:, :, bs],
                                     in1=src3[:, :, bs])
                # sum (partition 0) and sum-of-squares (partition 32)
                # share one PSUM bank via column tile groups.
                st = pstat.tile([33, 512], F32, name="st", tag="stat")
                for k in range(KC):
                    nc.tensor.matmul(st[0:1, 0:S], ones_col[:],
                                     src3[:, k, bs],
                                     start=(k == 0), stop=(k == KC - 1))
                    nc.tensor.matmul(st[32:33, 0:S], ones_col[:],
                                     sqb[:, k, :],
                                     start=(k == 0), stop=(k == KC - 1),
                                     tile_position=(0, 32))
                musq = rows.tile([1, S], F32, name="musq", tag="row")
                nc.scalar.activation(out=musq[0:1, :], in_=st[0:1, 0:S],
                                     func=AF.Square, scale=-1.0 / E)
                muneg_b = rows.tile([1, S], BF16, name="muneg_b", tag="row")
                nc.scalar.mul(out=muneg_b[0:1, :], in_=st[0:1, 0:S],
                              mul=-1.0 / E)
                var = rows.tile([1, S], F32, name="var", tag="row")
                nc.vector.scalar_tensor_tensor(
                    out=var[0:1, :], in0=st[32:33, 0:S], scalar=1.0 / E,
                    in1=musq[0:1, :], op0=ALU.mult, op1=ALU.subtract)
                sd = rows.tile([1, S], F32, name="sd", tag="row")
                nc.scalar.activation(out=sd[0:1, :], in_=var[0:1, :],
                                     func=AF.Sqrt, bias=eps_t[0:1, 0:1])
                rstd = rows.tile([1, S], F32, name="rstd", tag="row")
                nc.vector.reciprocal_approx_fast(out=rstd[0:1, :],
                                                 in_=sd[0:1, :])
                rstd_b = rows.tile([1, S], BF16, name="rstd_b", tag="row")
                nc.scalar.mul(out=rstd_b[0:1, :], in_=rstd[0:1, :], mul=S_X)
                psA = pbc.tile([128, 1, 512], F32, name="psA", tag="bc")
                psB = pbc.tile([128, 1, 512], F32, name="psB", tag="bc")
                nc.tensor.matmul(psA[:, 0, 0:S], ones_row[0:1, :],
                                 rstd_b[0:1, :], start=True, stop=True)
                nc.tensor.matmul(psB[:, 0, 0:S], ones_row[0:1, :],
                                 muneg_b[0:1, :], start=True, stop=True)
                if spin:
                    emit_spin(spin_t, spin)
                # fused normalize: all 8 k-slices per DVE op via stride-0
                # broadcast of the mean/rstd rows
                tmp = lntmp.tile([128, KC, S], tmp_dt, name="tmp", tag="ap")
                nc.vector.tensor_add(
                    out=tmp[:], in0=apply3[:, :, bs],
                    in1=psB[:, 0:1, 0:S].broadcast_to((128, KC, S)))
                nc.vector.tensor_mul(
                    out=out3[b][:, :, 0:S], in0=tmp[:],
                    in1=psA[:, 0:1, 0:S].broadcast_to((128, KC, S)))
                if after_b is not None:
                    after_b(b)

        with tc.tile_pool(name="ctx3", bufs=B_LOC) as ctx_p:
            ctx3 = [ctx_p.tile([128, MC_E, SP], FP8, tag="ctx3", name="ctx3")
                    for _ in range(B_LOC)]

            # ============= LN1 (+V interleaved per batch) ===============
            with tc.tile_pool(name="x3", bufs=B_LOC) as x3_p, \
                    tc.tile_pool(name="vpool", bufs=2 * B_LOC + 1) as v_p:
                x3 = [x3_p.tile([128, KC, SP], FP8, tag="x3", name="x3")
                      for _ in range(B_LOC)]
                v_tiles = {}
                with ExitStack() as ln1_ph:
                    vw_p = ln1_ph.enter_context(tc.tile_pool(name="vw", bufs=4))
                    xtb_p = ln1_ph.enter_context(
                        tc.tile_pool(name="xtb", bufs=1))
                    ppv = ln1_ph.enter_context(
                        tc.tile_pool(name="ppv", bufs=2, space="PSUM"))
                    spin_p1 = ln1_ph.enter_context(
                        tc.tile_pool(name="spin1", bufs=1, space="PSUM"))
                    spin_t1 = spin_p1.tile([1, 512], F32, name="spin",
                                           tag="spin")

                    xTb3 = xtb_p.tile([128, KC, NT], BF16, name="xTb3",
                                      tag="xTb3")
                    vw_sb = []
                    for b in range(B_LOC):
                        for k in range(KC):
                            nc.sync.dma_start(
                                out=xTb3[:, k, b * S:(b + 1) * S],
                                in_=xTb_d[k * 128:(k + 1) * 128,
                                          b * S:(b + 1) * S])
                        if b == 0:
                            for kp in range(KC // 2):
                                vwk = vw_p.tile([128, 2, E], FP8, name="vwk",
                                                tag="vwk")
                                nc.sync.dma_start(out=vwk[:],
                                                  in_=vw_d[kp, :, :, :])
                                vw_sb.append(vwk)

                    emit_spin(spin_t1, 96)

                    def v_proj(b):
                        for jc, (j0, jcs) in enumerate(JC[:2]):
                            ps = ppv.tile([128, 2, 512], F32,
                                          name="vps", tag="vps")
                            for kp in range(KC // 2):
                                for n in range(2):
                                    nc.tensor.matmul(
                                        ps[0:jcs, n, :],
                                        x3[b][:, 2 * kp:2 * kp + 2,
                                              j0:j0 + jcs],
                                        vw_sb[kp][:, :, n * 512:(n + 1) * 512],
                                        start=(kp == 0), stop=(kp == 3),
                                        perf_mode=DR)
                            # [tok, H, 128]: cols 0:64 hold 1/S_CTX, cols
                            # 64:128 V -> ctx matmul replicates the softmax
                            # sums (pre-scaled for fp8 ctx) across partitions.
                            vt = v_p.tile([128, H, 128], BF16,
                                          name="vt", tag="vt")
                            if with_vbias:
                                nc.vector.scalar_tensor_tensor(
                                    out=vt[0:jcs, :, 64:128],
                                    in0=ps[0:jcs, :, :], scalar=dq_v,
                                    in1=vb_sb[0:jcs, :],
                                    op0=ALU.mult, op1=ALU.add)
                            elif jc == 0:
                                nc.scalar.mul(out=vt[0:jcs, :, 64:128],
                                              in_=ps[0:jcs, :, :], mul=dq_v)
                            else:
                                nc.vector.tensor_scalar_mul(
                                    out=vt[0:jcs, :, 64:128],
                                    in0=ps[0:jcs, :, :], scalar1=dq_v)
                            nc.gpsimd.memset(vt[:, :, 0:64], 1.0 / S_CTX)
                            v_tiles[(b, jc)] = vt

                    emit_ln(ln1_ph, xTb3, x3, "1", spin_t=spin_t1, spin=24,
                            after_b=v_proj)

                    # the 4 batches' tail token (j=256): pack the M=1
                    # matmuls into column groups 0/32/64/96 (fp8, normal
                    # mode) so they run concurrently on the PE array.
                    ps = ppv.tile([128, 2, 512], F32, name="vps_t", tag="vps")
                    for n in range(2):
                        for k in range(KC):
                            for b in range(B_LOC):
                                nc.tensor.matmul(
                                    ps[32 * b:32 * b + 1, n, :],
                                    x3[b][:, k, 256:257],
                                    vw_sb[k // 2][:, k % 2,
                                                  n * 512:(n + 1) * 512],
                                    start=(k == 0), stop=(k == KC - 1),
                                    tile_position=(0, 32 * b))
                    vt_t = v_p.tile([128, H, 128], BF16, name="vt_t", tag="vt")
                    nc.gpsimd.memset(vt_t[:, :, 0:64], 1.0 / S_CTX)
                    for b in range(B_LOC):
                        if with_vbias:
                            nc.vector.scalar_tensor_tensor(
                                out=vt_t[32 * b:32 * b + 1, :, 64:128],
                                in0=ps[32 * b:32 * b + 1, :, :], scalar=dq_v,
                                in1=vb_sb[0:1, :], op0=ALU.mult, op1=ALU.add)
                        else:
                            nc.scalar.mul(
                                out=vt_t[32 * b:32 * b + 1, :, 64:128],
                                in_=ps[32 * b:32 * b + 1, :, :], mul=dq_v)
                        v_tiles[(b, 2)] = vt_t

                # ========= Q/K + attention (per head-pair chunk) =====
                with ExitStack() as ph:
                    qt_p = ph.enter_context(tc.tile_pool(name="qt", bufs=2))
                    kt_p = ph.enter_context(tc.tile_pool(name="kt", bufs=2))
                    wqk_p = ph.enter_context(
                        tc.tile_pool(name="wqk", bufs=6))
                    e_p = ph.enter_context(tc.tile_pool(name="ep", bufs=9))
                    rs_p = ph.enter_context(tc.tile_pool(name="rsp", bufs=4))
                    if with_mask:
                        msk_p = ph.enter_context(
                            tc.tile_pool(name="mskp", bufs=3 * B_LOC))
                    pp2 = ph.enter_context(
                        tc.tile_pool(name="pp2", bufs=3, space="PSUM"))
                    psp = ph.enter_context(
                        tc.tile_pool(name="psp", bufs=2, space="PSUM"))
                    spin_pa = ph.enter_context(
                        tc.tile_pool(name="spina", bufs=1, space="PSUM"))
                    spin_ta = spin_pa.tile([1, 512], F32, name="spin",
                                           tag="spin")
                    if with_mask:
                        msk = {}
                        for b in range(B_LOC):
                            for jc, (j0, jcs) in enumerate(JC):
                                mt = msk_p.tile([128, S], F32, name="mt",
                                                tag="mt")
                                nc.sync.dma_start(
                                    out=mt[0:jcs, :],
                                    in_=mskT_d[b, j0:j0 + jcs, :])
                                msk[(b, jc)] = mt

                    for ec in range(MC_E):
                        qkt = []
                        for (w_d, b_sb, opool) in (
                                (qw_d, qb_sb, qt_p),
                                (kw_d, kb_sb, kt_p)):
                            wt = wqk_p.tile([128, KC, 128], FP8,
                                            name="wqk", tag="wqk")
                            nc.sync.dma_start(out=wt[:],
                                              in_=w_d[ec, :, :, :])
                            ot = opool.tile([128, NT], BF16,
                                            name="qk", tag="qk")
                            for half in range(2):
                                pss = [pp2.tile([128, 512], F32,
                                                name="pqk", tag="pqk")
                                       for _ in range(2)]
                                for kp in range(KC // 2):
                                    for bb in range(2):
                                        b = half * 2 + bb
                                        nc.tensor.matmul(
                                            pss[bb][:, 0:S],
                                            wt[:, 2 * kp:2 * kp + 2, :],
                                            x3[b][:, 2 * kp:2 * kp + 2, 0:S],
                                            start=(kp == 0), stop=(kp == 3),
                                            perf_mode=DR)
                                for bb in range(2):
                                    b = half * 2 + bb
                                    if with_qkbias:
                                        nc.vector.tensor_scalar_add(
                                            out=ot[:, b * S:(b + 1) * S],
                                            in0=pss[bb][:, 0:S],
                                            scalar1=b_sb[:, ec:ec + 1])
                                    else:
                                        nc.vector.tensor_copy(
                                            out=ot[:, b * S:(b + 1) * S],
                                            in_=pss[bb][:, 0:S])
                            qkt.append(ot)
                        qte, kte = qkt

                        # tail key (j=256) for all 4 batches: packed into
                        # array col groups 32b / row groups 64*hi, one
                        # shared per-hi exp over all rows.
                        ps_t = [pp2.tile([128, 512], F32, name="ps_t",
                                         tag="pqk") for _ in range(2)]
                        et_t = [e_p.tile([128, S], BF16, name="et_t",
                                         tag="et") for _ in range(2)]
                        for hi in range(2):
                            p0 = hi * 64
                            for b in range(B_LOC):
                                nc.tensor.matmul(
                                    ps_t[hi][32 * b:32 * b + 1, 0:S],
                                    kte[p0:p0 + 64,
                                        b * S + 256: b * S + 257],
                                    qte[p0:p0 + 64, b * S:(b + 1) * S],
                                    start=True, stop=True,
                                    tile_position=(p0, 32 * b))
                            if with_mask:
                                for b in range(B_LOC):
                                    nc.vector.tensor_add(
                                        out=ps_t[hi][32 * b:32 * b + 1, 0:S],
                                        in0=ps_t[hi][32 * b:32 * b + 1, 0:S],
                                        in1=msk[(b, 2)][0:1, :])
                            nc.scalar.activation(out=et_t[hi][0:97, :],
                                                 in_=ps_t[hi][0:97, 0:S],
                                                 func=AF.Exp, scale=dq_qk)

                        for b in range(B_LOC):
                            ets = []
                            for jc, (j0, jcs) in enumerate(JC[:2]):
                                sp = psp.tile([128, 2, 512], F32,
                                              name="sp", tag="sp")
                                for hi in range(2):
                                    p0 = hi * 64
                                    nc.tensor.matmul(
                                        sp[0:jcs, hi, 0:S],
                                        kte[p0:p0 + 64,
                                            b * S + j0: b * S + j0 + jcs],
                                        qte[p0:p0 + 64,
                                            b * S:(b + 1) * S],
                                        start=True, stop=True)
                                if with_mask:
                                    for hi in range(2):
                                        nc.vector.tensor_add(
                                            out=sp[0:jcs, hi, 0:S],
                                            in0=sp[0:jcs, hi, 0:S],
                                            in1=msk[(b, jc)][0:jcs, :])
                                et = e_p.tile([128, 2, S], BF16,
                                              name="et", tag="et2")
                                nc.scalar.activation(
                                    out=et[0:jcs, :, :],
                                    in_=sp[0:jcs, :, 0:S], func=AF.Exp,
                                    scale=dq_qk)
                                ets.append(et)
                            cp = psp.tile([128, 2, 512], F32,
                                          name="cp", tag="sp")
                            for hi in range(2):
                                h = 2 * ec + hi
                                for jc, (j0, jcs) in enumerate(JC[:2]):
                                    nc.tensor.matmul(
                                        cp[0:128, hi, 0:S],
                                        v_tiles[(b, jc)][0:jcs, h, :],
                                        ets[jc][0:jcs, hi, :],
                                        start=(jc == 0), stop=False)
                                nc.tensor.matmul(
                                    cp[0:128, hi, 0:S],
                                    v_tiles[(b, 2)][32 * b:32 * b + 1, h, :],
                                    et_t[hi][32 * b:32 * b + 1, :],
                                    start=False, stop=True,
                                    tile_position=(32 * b, 0))
                            rst = rs_p.tile([64, 2, S], F32,
                                            name="rst", tag="rst")
                            nc.vector.reciprocal_approx_fast(
                                out=rst[0:64, :, :],
                                in_=cp[0:64, :, 0:S])
                            for hi in range(2):
                                nc.vector.tensor_mul(
                                    out=ctx3[b][hi * 64:hi * 64 + 64,
                                                ec, 0:S],
                                    in0=cp[64:128, hi, 0:S],
                                    in1=rst[0:64, hi, :])
                            emit_spin(spin_ta, 6)

            # ============= out projection + residual =================
            ht_p = top.enter_context(
                tc.tile_pool(name="ht3", bufs=1, side="right"))
            ht3 = ht_p.tile([128, KC, NT], F32, name="ht3", tag="ht3")
            htb_p = top.enter_context(
                tc.tile_pool(name="htb3", bufs=1, side="right"))
            htb3 = htb_p.tile([128, KC, NT], BF16, name="htb3", tag="htb3")
            with ExitStack() as ph:
                wo_p = ph.enter_context(tc.tile_pool(name="wo", bufs=5))
                xt_p = ph.enter_context(
                    tc.tile_pool(name="xt", bufs=12))
                ppo = ph.enter_context(
                    tc.tile_pool(name="ppo", bufs=2, space="PSUM"))
                for m in range(MC_E):
                    wt = wo_p.tile([128, KC, 128], FP8, name="wo", tag="wo")
                    nc.sync.dma_start(out=wt[:], in_=ow_d[m, :, :, :])
                    xt = []
                    for b in range(B_LOC):
                        t = xt_p.tile([128, S], F32, name="xt", tag="xt")
                        nc.sync.dma_start(
                            out=t[:],
                            in_=xT_d[m * 128:(m + 1) * 128,
                                     b * S:(b + 1) * S])
                        xt.append(t)
                    ps = ppo.tile([128, B_LOC, 512], F32, name="po",
                                  tag="po")
                    for kp in range(KC // 2):
                        for b in range(B_LOC):
                            nc.tensor.matmul(
                                ps[:, b, 0:S],
                                wt[:, 2 * kp:2 * kp + 2, :],
                                ctx3[b][:, 2 * kp:2 * kp + 2, 0:S],
                                start=(kp == 0), stop=(kp == 3),
                                perf_mode=DR)
                    for b in range(B_LOC):
                        bs = slice(b * S, (b + 1) * S)
                        if b < 2:
                            nc.vector.scalar_tensor_tensor(
                                out=ht3[:, m, bs], in0=ps[:, b, 0:S],
                                scalar=dq_o, in1=xt[b][:],
                                op0=ALU.mult, op1=ALU.add)
                        else:
                            # ACT dequants from PSUM, GpSimd adds residual
                            # (GpSimd cannot read PSUM)
                            to = xt_p.tile([128, S], F32, name="to", tag="xt")
                            nc.scalar.mul(out=to[:], in_=ps[:, b, 0:S],
                                          mul=dq_o)
                            nc.gpsimd.tensor_add(out=ht3[:, m, bs],
                                                 in0=to[:], in1=xt[b][:])
                        if with_obias:
                            nc.vector.tensor_scalar_add(
                                out=ht3[:, m, bs], in0=ht3[:, m, bs],
                                scalar1=ob_sb[:, m:m + 1])
                    nc.gpsimd.tensor_copy(out=htb3[:, m, :],
                                          in_=ht3[:, m, :])
        # ctx3 closed

        # ================= LN2 + MLP =====================================
        with tc.tile_pool(name="xln2", bufs=B_LOC) as xln2_p:
            xln2_3 = [xln2_p.tile([128, KC, SP], FP8, tag="x3b", name="x3b")
                      for _ in range(B_LOC)]
            f1o_p = top.enter_context(
                tc.tile_pool(name="f1o", bufs=MC_I, side="right"))
            f1o = []
            with ExitStack() as ln2_ph:
                spin_p2 = ln2_ph.enter_context(
                    tc.tile_pool(name="spin2", bufs=1, space="PSUM"))
                spin_t2 = spin_p2.tile([1, 512], F32, name="spin", tag="spin")
                emit_ln(ln2_ph, htb3, xln2_3, "2", apply3=ht3,
                        spin_t=spin_t2, spin=24)
                wf1_p = ln2_ph.enter_context(tc.tile_pool(name="wf1", bufs=6))
                ppf1 = ln2_ph.enter_context(
                    tc.tile_pool(name="ppf1", bufs=2, space="PSUM"))
                for m in range(MC_I):
                    wt = wf1_p.tile([128, KC, 128], FP8, name="wf1",
                                    tag="wf1")
                    nc.sync.dma_start(out=wt[:], in_=f1w_d[m, :, :, :])
                    o = f1o_p.tile([128, NT], BF16, name="f1o", tag="f1o")
                    for half in range(2):
                        ps = ppf1.tile([128, 2, 512], F32, name="pf1",
                                       tag="pf1")
                        for kp in range(KC // 2):
                            for bb in range(2):
                                b = half * 2 + bb
                                nc.tensor.matmul(
                                    ps[:, bb, 0:S],
                                    wt[:, 2 * kp:2 * kp + 2, :],
                                    xln2_3[b][:, 2 * kp:2 * kp + 2, 0:S],
                                    start=(kp == 0), stop=(kp == 3),
                                    perf_mode=DR)
                        nc.scalar.activation(
                            out=o[:, half * 2 * S:(half + 1) * 2 * S],
                            in_=ps[:, :, 0:S],
                            func=AF.Gelu_apprx_tanh,
                            bias=f1b_sb[:, m:m + 1],
                            scale=dq_f1)
                    f1o.append(o)

        with ExitStack() as ph:
            wf2_p = ph.enter_context(tc.tile_pool(name="wf2", bufs=3))
            ppf2 = ph.enter_context(
                tc.tile_pool(name="ppf2", bufs=2, space="PSUM"))
            out_p = ph.enter_context(tc.tile_pool(name="outp", bufs=3))
            for m in range(MC_E):
                wt = wf2_p.tile([128, MC_I, 128], BF16, name="wf2", tag="wf2")
                nc.sync.dma_start(out=wt[:], in_=f2w_d[m, :, :, :])
                ps = ppf2.tile([128, B_LOC, 512], F32, name="pf2", tag="pf2")
                for b in range(B_LOC):
                    for k in range(MC_I):
                        nc.tensor.matmul(
                            ps[:, b, 0:S], wt[:, k, :],
                            f1o[k][:, b * S:(b + 1) * S],
                            start=(k == 0), stop=(k == MC_I - 1))
                o = out_p.tile([128, NT], F32, name="oo", tag="oo")
                nc.vector.scalar_tensor_tensor(
                    out=o[:], in0=ps[:, :, 0:S], scalar=f2b_sb[:, m:m + 1],
                    in1=ht3[:, m, :], op0=ALU.add, op1=ALU.add)
                nc.sync.dma_start(out=outT_d[m * 128:(m + 1) * 128, :],
                                  in_=o[:])

    nc.compile()
    return nc


FP8_NP = ml_dtypes.float8_e4m3fn


def _q8(W, s):
    """Quantize W*s to e4m3 (clipped to TRN max normal 240)."""
    return np.clip(np.asarray(W, np.float32) * s, -240, 240).astype(FP8_NP)


def _pack_lhsT8(W, s):
    """W [M, K] (out, in) -> [M/128, 128, K/128, 128] fp8 with
    [m, p, k, j] = W[m*128+j, k*128+p]*s (lhsT tiles, partition = K)."""
    W = np.asarray(W, np.float32)
    M, K = W.shape
    A = W.reshape(M // 128, 128, K // 128, 128)
    return _q8(np.ascontiguousarray(A.transpose(0, 3, 2, 1)), s)


def _pack_lhsT(W):
    """bf16 variant of _pack_lhsT8 (no scale)."""
    W = np.asarray(W, np.float32)
    M, K = W.shape
    A = W.reshape(M // 128, 128, K // 128, 128)
    return np.ascontiguousarray(A.transpose(0, 3, 2, 1)).astype(ml_dtypes.bfloat16)


def _pack_pbias(b):
    """b [M] -> [128, M/128] f32 per-partition bias columns."""
    return np.ascontiguousarray(np.asarray(b, np.float32).reshape(-1, 128).T)


def _wscale(W):
    return float(120.0 / max(np.abs(np.asarray(W, np.float32)).max(), 1e-30))


def kernel(hidden_states, attention_mask, causal_attention_mask,
           ln1_w, ln1_b, q_w, q_b, k_w, k_b, v_w, v_b, o_w, o_b,
           ln2_w, ln2_b, fc1_w, fc1_b, fc2_w, fc2_b):
    global LAST_EXEC_NS
    from concourse.bass_utils import run_bass_kernel_spmd

    hs = np.asarray(hidden_states, np.float32)
    msk = (np.asarray(attention_mask, np.float32)
           + np.asarray(causal_attention_mask, np.float32))
    with_mask = bool(np.any(msk))

    ln1_w = np.asarray(ln1_w, np.float32); ln1_b = np.asarray(ln1_b, np.float32)
    ln2_w = np.asarray(ln2_w, np.float32); ln2_b = np.asarray(ln2_b, np.float32)
    q_w = np.asarray(q_w, np.float32); q_b = np.asarray(q_b, np.float32)
    k_w = np.asarray(k_w, np.float32); k_b = np.asarray(k_b, np.float32)
    v_w = np.asarray(v_w, np.float32); v_b = np.asarray(v_b, np.float32)
    o_w = np.asarray(o_w, np.float32); o_b = np.asarray(o_b, np.float32)
    fc1_w = np.asarray(fc1_w, np.float32); fc1_b = np.asarray(fc1_b, np.float32)
    fc2_w = np.asarray(fc2_w, np.float32); fc2_b = np.asarray(fc2_b, np.float32)

    scale = D ** -0.5
    # fold LN1 scale/bias into Q/K/V, and the softmax scale into Q
    qw_eff = (q_w * ln1_w[None, :]) * scale
    qb_eff = (q_b + q_w @ ln1_b) * scale
    kw_eff = k_w * ln1_w[None, :]
    kb_eff = k_b + k_w @ ln1_b
    vw_eff = v_w * ln1_w[None, :]
    vb_eff = v_b + v_w @ ln1_b
    # fold LN2 into fc1
    f1w_eff = fc1_w * ln2_w[None, :]
    f1b_eff = fc1_b + fc1_w @ ln2_b

    # fp8 weight scales (LN activations are pre-scaled by S_X on device)
    s_wq = _wscale(qw_eff)
    s_wk = _wscale(kw_eff)
    s_wv = _wscale(vw_eff)
    s_wo = _wscale(o_w)
    s_wf1 = _wscale(f1w_eff)
    dq_qk = 1.0 / (S_X * S_X * s_wq * s_wk)
    dq_v = 1.0 / (S_X * s_wv)
    dq_o = 1.0 / (S_CTX * s_wo)
    dq_f1 = 1.0 / (S_X * s_wf1)

    # vw: [E_in, E_out] grouped into k-pairs -> [KC/2, 128, 2, E] fp8
    vw_t = np.ascontiguousarray(vw_eff.T.reshape(KC, 128, E))
    vw_pk = np.ascontiguousarray(
        vw_t.reshape(KC // 2, 2, 128, E).transpose(0, 2, 1, 3))

    base = {
        "qw": _pack_lhsT8(qw_eff, s_wq),
        "kw": _pack_lhsT8(kw_eff, s_wk),
        "vw": _q8(vw_pk, s_wv),
        "ow": _pack_lhsT8(o_w, s_wo),
        "f1w": _pack_lhsT8(f1w_eff, s_wf1),
        "f2w": _pack_lhsT(fc2_w),
        "qb": _pack_pbias(qb_eff * (S_X * s_wq)),
        "kb": _pack_pbias(kb_eff * (S_X * s_wk)),
        "vb": np.ascontiguousarray(vb_eff[None, :].astype(np.float32)),
        "ob": _pack_pbias(o_b),
        "f1b": _pack_pbias(f1b_eff),
        "f2b": _pack_pbias(fc2_b),
    }

    with_vbias = bool(np.any(vb_eff))
    with_qkbias = bool(np.any(qb_eff)) or bool(np.any(kb_eff))
    with_obias = bool(np.any(o_b))
    key = (with_mask, with_vbias, with_qkbias, with_obias,
           dq_v, dq_qk, dq_o, dq_f1)
    if key not in _cache:
        _cache[key] = _build(with_mask, with_vbias, with_qkbias, with_obias,
                             dq_v, dq_qk, dq_o, dq_f1)
    nc = _cache[key]

    in_maps = []
    for c in range(N_CORES):
        x = hs[c * B_LOC:(c + 1) * B_LOC].reshape(NT, E).T
        m = dict(base)
        m["xT"] = np.ascontiguousarray(x)
        m["xTb"] = np.ascontiguousarray(x).astype(ml_dtypes.bfloat16)
        if with_mask:
            m["mskT"] = np.ascontiguousarray(
                msk[c * B_LOC:(c + 1) * B_LOC, 0].transpose(0, 2, 1)
                / dq_qk)
        in_maps.append(m)

    res = run_bass_kernel_spmd(nc, in_maps, core_ids=list(range(N_CORES)),
                               trace=TRACE)
    LAST_EXEC_NS = res.exec_time_ns

    outs = []
    for c in range(N_CORES):
        oT = res.results[c]["outT"]          # [E, NT] f32
        outs.append(np.ascontiguousarray(oT.T).reshape(B_LOC, S, E))
    return np.concatenate(outs, axis=0)


# revision 34
# speedup vs baseline: 1.1074x; 1.1074x over previous
"""Trainium2 Bass kernel for a CLIP encoder layer (B=32, S=257, E=1024, H=16, I=4096).

Strategy: data-parallel over batch across 8 NeuronCores (4 batch elements per
core), no collectives.  Per-core compute is feature-major ([E, tokens]).

Quantization: Q/K/V/O/fc1 matmuls run in fp8(e4m3) with DoubleRow perf mode
(2 k-subtiles contracted per PE pass, 2x bf16 throughput); fc2 stays bf16
(fp8 there would blow the error budget).  Per-tensor scales fold into the
weights host-side; dequant scales fold into existing PSUM->SBUF ops (exp /
gelu / output adds), so no extra passes.

  - Activations live in 3D tiles: xT3/ht3 [128, KC, NT] f32 (LN source +
    residual), x3/xln2_3/ctx3 [128, KC, SP] fp8 whose k-pair slices feed
    DoubleRow matmuls directly.
  - LayerNorm: column sums via f32r ones-matmul straight on the f32 source,
    sum-of-squares via bf16 ones-matmul on an ACT-squared copy (both stats
    share one PSUM bank via column tile groups); mean/rstd broadcast via PE;
    the normalize runs as fused 4-k-wide DVE ops with stride-0 broadcast.
  - Q/K/O/fc1: weight-stationary DoubleRow matmuls, k-pair-outer loops so
    each LDWEIGHTS serves 2-4 matmuls.
  - V: activation-stationary DoubleRow -> token-major [tok, H, 128] tiles
    with a 1/S_CTX column so softmax denominators fall out of the ctx matmul
    (and ctx lands pre-scaled for fp8).
  - Attention: scores transposed, softmax over partition dim via ctx matmul,
    exp on ACT from PSUM with qk dequant folded into its scale.  All
    attention PSUM tiles are single-bank [128, 512] with deep pool rotation
    (pp2 x3 + psp x5) so score/exp/ctx chains pipeline across batches.
  - PE warmth: V interleaves with LN1 per batch; spinner matmuls fill
    DVE/ACT-heavy windows so the HAM clock gate stays at 8/8.
"""

import numpy as np
import ml_dtypes

B, S, E, H, D, II = 32, 257, 1024, 16, 64, 4096
N_CORES = 8
B_LOC = B // N_CORES          # 4
NT = B_LOC * S                # 1028
KC = E // 128                 # 8
MC_E = E // 128               # 8
MC_I = II // 128              # 32
EPS = 1e-5
SP = 272                      # S padded so fp8 k-slice stride is 16B-aligned

S_X = 16.0                    # LN output fp8 scale
S_CTX = 128.0                 # ctx fp8 scale

# j-chunks of one batch element's 257 keys
JC = [(0, 128), (128, 128), (256, 1)]

TRACE = False
LAST_EXEC_NS = None

_cache = {}


def _build(with_mask, with_vbias, with_qkbias, with_obias,
           dq_v, dq_qk, dq_o, dq_f1):
    import concourse.tile as tile
    from concourse import bacc, mybir
    from contextlib import ExitStack

    F32 = mybir.dt.float32
    F32R = mybir.dt.float32r
    BF16 = mybir.dt.bfloat16
    FP8 = mybir.dt.float8e4
    AF = mybir.ActivationFunctionType
    ALU = mybir.AluOpType
    DR = mybir.MatmulPerfMode.DoubleRow

    nc = bacc.Bacc("TRN2", target_bir_lowering=False, debug=False,
                   enable_asserts=False, num_devices=N_CORES)

    xT_d = nc.dram_tensor("xT", [E, NT], F32, kind="ExternalInput")
    xTb_d = nc.dram_tensor("xTb", [E, NT], BF16, kind="ExternalInput")
    qw_d = nc.dram_tensor("qw", [MC_E, 128, KC, 128], FP8, kind="ExternalInput")
    kw_d = nc.dram_tensor("kw", [MC_E, 128, KC, 128], FP8, kind="ExternalInput")
    vw_d = nc.dram_tensor("vw", [KC // 2, 128, 2, E], FP8, kind="ExternalInput")
    ow_d = nc.dram_tensor("ow", [MC_E, 128, KC, 128], FP8, kind="ExternalInput")
    f1w_d = nc.dram_tensor("f1w", [MC_I, 128, KC, 128], FP8, kind="ExternalInput")
    f2w_d = nc.dram_tensor("f2w", [MC_E, 128, MC_I, 128], BF16, kind="ExternalInput")
    qb_d = nc.dram_tensor("qb", [128, MC_E], F32, kind="ExternalInput")
    kb_d = nc.dram_tensor("kb", [128, MC_E], F32, kind="ExternalInput")
    vb_d = nc.dram_tensor("vb", [1, E], F32, kind="ExternalInput")
    ob_d = nc.dram_tensor("ob", [128, MC_E], F32, kind="ExternalInput")
    f1b_d = nc.dram_tensor("f1b", [128, MC_I], F32, kind="ExternalInput")
    f2b_d = nc.dram_tensor("f2b", [128, MC_E], F32, kind="ExternalInput")
    mskT_d = None
    if with_mask:
        mskT_d = nc.dram_tensor("mskT", [B_LOC, S, S], F32, kind="ExternalInput")
    outT_d = nc.dram_tensor("outT", [E, NT], F32, kind="ExternalOutput")

    with tile.TileContext(nc) as tc, ExitStack() as top:
        consts = top.enter_context(tc.tile_pool(name="consts", bufs=1))

        ones_col = consts.tile([128, 1], BF16)
        nc.vector.memset(ones_col[:], 1.0)
        ones_row = consts.tile([1, 128], BF16)
        nc.vector.memset(ones_row[:], 1.0)
        eps_t = consts.tile([1, 1], F32)
        nc.vector.memset(eps_t[:], EPS)
        spin_x = consts.tile([128, 64], BF16)
        nc.vector.memset(spin_x[:], 0.0)
        qb_sb = consts.tile([128, MC_E], F32)
        nc.scalar.dma_start(out=qb_sb[:], in_=qb_d[:])
        kb_sb = consts.tile([128, MC_E], F32)
        nc.scalar.dma_start(out=kb_sb[:], in_=kb_d[:])
        ob_sb = consts.tile([128, MC_E], F32)
        nc.scalar.dma_start(out=ob_sb[:], in_=ob_d[:])
        f2b_sb = consts.tile([128, MC_E], F32)
        nc.scalar.dma_start(out=f2b_sb[:], in_=f2b_d[:])
        f1b_sb = consts.tile([128, MC_I], F32)
        nc.scalar.dma_start(out=f1b_sb[:], in_=f1b_d[:])
        vb_sb = None
        if with_vbias:
            vb_sb = consts.tile([128, E], F32)
            nc.scalar.dma_start(out=vb_sb[:],
                                in_=vb_d[0:1, :].to_broadcast((128, E)))



        def emit_spin(spin_t, n):
            """Dependency-free PE matmuls to keep the HAM clock gate warm
            through DVE/ACT-heavy windows."""
            for _ in range(n):
                nc.tensor.matmul(spin_t[0:1, 0:64], ones_col[:], spin_x[:],
                                 start=True, stop=True)

        def emit_ln(ph, src3, out3, sfx, apply3=None, spin_t=None, spin=0,
                    after_b=None):
            """Per-batch column LayerNorm over the feature (partition) dim.
            src3: [128, KC, NT] bf16 tile (stats input).  apply3: optional
            higher-precision source for the normalize (defaults to src3).
            Writes (x - mu) * rstd * S_X into out3[b][:, k, 0:S] (fp8); LN
            scale/bias are folded into the downstream weights host-side.
            after_b(b) emits follow-on work (e.g. V matmuls) interleaved."""
            if apply3 is None:
                apply3 = src3
            tmp_dt = BF16 if apply3 is src3 else F32
            lntmp = ph.enter_context(tc.tile_pool(name=f"lntmp{sfx}", bufs=2))
            sq_p = ph.enter_context(tc.tile_pool(name=f"sqp{sfx}", bufs=2))
            rows = ph.enter_context(tc.tile_pool(name=f"rows{sfx}", bufs=8))
            pstat = ph.enter_context(
                tc.tile_pool(name=f"pstat{sfx}", bufs=1, space="PSUM"))
            pbc = ph.enter_context(
                tc.tile_pool(name=f"pbc{sfx}", bufs=2, space="PSUM"))
            for b in range(B_LOC):
                bs = slice(b * S, (b + 1) * S)
                # squares on DVE (ACT is the scarcer engine here)
                sqb = sq_p.tile([128, KC, S], BF16, name="sqb", tag="sqb")
                nc.vector.tensor_mul(out=sqb[:], in0=src3[:, :, bs],
                                     in1=src3[:, :, bs])
                # sum (partition 0) and sum-of-squares (partition 32)
                # share one PSUM bank via column tile groups.
                st = pstat.tile([33, 512], F32, name="st", tag="stat")
                for k in range(KC):
                    nc.tensor.matmul(st[0:1, 0:S], ones_col[:],
                                     src3[:, k, bs],
                                     start=(k == 0), stop=(k == KC - 1))
                    nc.tensor.matmul(st[32:33, 0:S], ones_col[:],
                                     sqb[:, k, :],
                                     start=(k == 0), stop=(k == KC - 1),
                                     tile_position=(0, 32))
                musq = rows.tile([1, S], F32, name="musq", tag="row")
                nc.scalar.activation(out=musq[0:1, :], in_=st[0:1, 0:S],
                                     func=AF.Square, scale=-1.0 / E)
                muneg_b = rows.tile([1, S], BF16, name="muneg_b", tag="row")
                nc.scalar.mul(out=muneg_b[0:1, :], in_=st[0:1, 0:S],
                              mul=-1.0 / E)
                var = rows.tile([1, S], F32, name="var", tag="row")
                nc.vector.scalar_tensor_tensor(
                    out=var[0:1, :], in0=st[32:33, 0:S], scalar=1.0 / E,
                    in1=musq[0:1, :], op0=ALU.mult, op1=ALU.subtract)
                sd = rows.tile([1, S], F32, name="sd", tag="row")
                nc.scalar.activation(out=sd[0:1, :], in_=var[0:1, :],
                                     func=AF.Sqrt, bias=eps_t[0:1, 0:1])
                rstd = rows.tile([1, S], F32, name="rstd", tag="row")
                nc.vector.reciprocal_approx_fast(out=rstd[0:1, :],
                                                 in_=sd[0:1, :])
                rstd_b = rows.tile([1, S], BF16, name="rstd_b", tag="row")
                nc.scalar.mul(out=rstd_b[0:1, :], in_=rstd[0:1, :], mul=S_X)
                psA = pbc.tile([128, 1, 512], F32, name="psA", tag="bc")
                psB = pbc.tile([128, 1, 512], F32, name="psB", tag="bc")
                nc.tensor.matmul(psA[:, 0, 0:S], ones_row[0:1, :],
                                 rstd_b[0:1, :], start=True, stop=True)
                nc.tensor.matmul(psB[:, 0, 0:S], ones_row[0:1, :],
                                 muneg_b[0:1, :], start=True, stop=True)
                if spin:
                    emit_spin(spin_t, spin)
                # fused normalize: all 8 k-slices per DVE op via stride-0
                # broadcast of the mean/rstd rows
                tmp = lntmp.tile([128, KC, S], tmp_dt, name="tmp", tag="ap")
                nc.vector.tensor_add(
                    out=tmp[:], in0=apply3[:, :, bs],
                    in1=psB[:, 0:1, 0:S].broadcast_to((128, KC, S)))
                nc.vector.tensor_mul(
                    out=out3[b][:, :, 0:S], in0=tmp[:],
                    in1=psA[:, 0:1, 0:S].broadcast_to((128, KC, S)))
                if after_b is not None:
                    after_b(b)

        with tc.tile_pool(name="ctx3", bufs=B_LOC) as ctx_p:
            ctx3 = [ctx_p.tile([128, MC_E, SP], FP8, tag="ctx3", name="ctx3")
                    for _ in range(B_LOC)]

            # ============= LN1 (+V interleaved per batch) ===============
            with tc.tile_pool(name="x3", bufs=B_LOC) as x3_p, \
                    tc.tile_pool(name="vpool", bufs=2 * B_LOC + 1) as v_p:
                x3 = [x3_p.tile([128, KC, SP], FP8, tag="x3", name="x3")
                      for _ in range(B_LOC)]
                v_tiles = {}
                with ExitStack() as ln1_ph:
                    vw_p = ln1_ph.enter_context(tc.tile_pool(name="vw", bufs=4))
                    xtb_p = ln1_ph.enter_context(
                        tc.tile_pool(name="xtb", bufs=1))
                    ppv = ln1_ph.enter_context(
                        tc.tile_pool(name="ppv", bufs=2, space="PSUM"))
                    spin_p1 = ln1_ph.enter_context(
                        tc.tile_pool(name="spin1", bufs=1, space="PSUM"))
                    spin_t1 = spin_p1.tile([1, 512], F32, name="spin",
                                           tag="spin")

                    xTb3 = xtb_p.tile([128, KC, NT], BF16, name="xTb3",
                                      tag="xTb3")
                    vw_sb = []
                    for k in range(KC):
                        nc.sync.dma_start(
                            out=xTb3[:, k, :],
                            in_=xTb_d[k * 128:(k + 1) * 128, :])
                    for kp in range(KC // 2):
                        vwk = vw_p.tile([128, 2, E], FP8, name="vwk",
                                        tag="vwk")
                        nc.sync.dma_start(out=vwk[:], in_=vw_d[kp, :, :, :])
                        vw_sb.append(vwk)
                    xtail_p = ln1_ph.enter_context(
                        tc.tile_pool(name="xtail", bufs=1))
                    # tail-token lhsT with batches at columns 0/32/64/96 so
                    # the DR matmul lands them at row-group-aligned PSUM
                    # partitions (stride 112 keeps fp8 k-slices 16B-aligned)
                    xtail = xtail_p.tile([128, KC, 112], FP8, name="xtail",
                                         tag="xtail")
                    nc.vector.memset(xtail[:], 0.0)

                    emit_spin(spin_t1, 96)

                    def v_proj(b):
                        for jc, (j0, jcs) in enumerate(JC[:2]):
                            ps = ppv.tile([128, 2, 512], F32,
                                          name="vps", tag="vps")
                            for kp in range(KC // 2):
                                for n in range(2):
                                    nc.tensor.matmul(
                                        ps[0:jcs, n, :],
                                        x3[b][:, 2 * kp:2 * kp + 2,
                                              j0:j0 + jcs],
                                        vw_sb[kp][:, :, n * 512:(n + 1) * 512],
                                        start=(kp == 0), stop=(kp == 3),
                                        perf_mode=DR)
                            # [tok, H, 128]: cols 0:64 hold 1/S_CTX, cols
                            # 64:128 V -> ctx matmul replicates the softmax
                            # sums (pre-scaled for fp8 ctx) across partitions.
                            vt = v_p.tile([128, H, 128], BF16,
                                          name="vt", tag="vt")
                            if with_vbias:
                                nc.vector.scalar_tensor_tensor(
                                    out=vt[0:jcs, :, 64:128],
                                    in0=ps[0:jcs, :, :], scalar=dq_v,
                                    in1=vb_sb[0:jcs, :],
                                    op0=ALU.mult, op1=ALU.add)
                            elif jc == 0:
                                nc.scalar.mul(out=vt[0:jcs, :, 64:128],
                                              in_=ps[0:jcs, :, :], mul=dq_v)
                            else:
                                nc.vector.tensor_scalar_mul(
                                    out=vt[0:jcs, :, 64:128],
                                    in0=ps[0:jcs, :, :], scalar1=dq_v)
                            nc.gpsimd.memset(vt[:, :, 0:64], 1.0 / S_CTX)
                            v_tiles[(b, jc)] = vt
                        nc.vector.tensor_copy(
                            out=xtail[:, :, 32 * b:32 * b + 1],
                            in_=x3[b][:, :, 256:257])

                    emit_ln(ln1_ph, xTb3, x3, "1", spin_t=spin_t1, spin=24,
                            after_b=v_proj)

                    # the 4 batches' tail token (j=256): pack the M=1
                    # matmuls into column groups 0/32/64/96 (fp8, normal
                    # mode) so they run concurrently on the PE array.
                    ps = ppv.tile([128, 2, 512], F32, name="vps_t", tag="vps")
                    for kp in range(KC // 2):
                        for n in range(2):
                            nc.tensor.matmul(
                                ps[0:97, n, :],
                                xtail[:, 2 * kp:2 * kp + 2, 0:97],
                                vw_sb[kp][:, :, n * 512:(n + 1) * 512],
                                start=(kp == 0), stop=(kp == 3),
                                perf_mode=DR)
                    vt_t = v_p.tile([128, H, 128], BF16, name="vt_t", tag="vt")
                    nc.gpsimd.memset(vt_t[:, :, 0:64], 1.0 / S_CTX)
                    if with_vbias:
                        nc.vector.scalar_tensor_tensor(
                            out=vt_t[0:97, :, 64:128],
                            in0=ps[0:97, :, :], scalar=dq_v,
                            in1=vb_sb[0:97, :], op0=ALU.mult, op1=ALU.add)
                    else:
                        nc.vector.tensor_scalar_mul(
                            out=vt_t[0:97, :, 64:128],
                            in0=ps[0:97, :, :], scalar1=dq_v)
                    for b in range(B_LOC):
                        v_tiles[(b, 2)] = vt_t

                # ========= Q/K + attention (per head-pair chunk) =====
                with ExitStack() as ph:
                    qt_p = ph.enter_context(tc.tile_pool(name="qt", bufs=2))
                    kt_p = ph.enter_context(tc.tile_pool(name="kt", bufs=2))
                    wqk_p = ph.enter_context(
                        tc.tile_pool(name="wqk", bufs=6))
                    e_p = ph.enter_context(tc.tile_pool(name="ep", bufs=9))
                    rs_p = ph.enter_context(tc.tile_pool(name="rsp", bufs=4))
                    if with_mask:
                        msk_p = ph.enter_context(
                            tc.tile_pool(name="mskp", bufs=3 * B_LOC))
                    pp2 = ph.enter_context(
                        tc.tile_pool(name="pp2", bufs=2, space="PSUM"))
                    psp = ph.enter_context(
                        tc.tile_pool(name="psp", bufs=3, space="PSUM"))
                    if with_mask:
                        msk = {}
                        for b in range(B_LOC):
                            for jc, (j0, jcs) in enumerate(JC):
                                mt = msk_p.tile([128, S], F32, name="mt",
                                                tag="mt")
                                nc.sync.dma_start(
                                    out=mt[0:jcs, :],
                                    in_=mskT_d[b, j0:j0 + jcs, :])
                                msk[(b, jc)] = mt

                    for ec in range(MC_E):
                        qkt = []
                        for (w_d, b_sb, opool) in (
                                (qw_d, qb_sb, qt_p),
                                (kw_d, kb_sb, kt_p)):
                            wt = wqk_p.tile([128, KC, 128], FP8,
                                            name="wqk", tag="wqk")
                            nc.sync.dma_start(out=wt[:],
                                              in_=w_d[ec, :, :, :])
                            ot = opool.tile([128, NT], BF16,
                                            name="qk", tag="qk")
                            for half in range(2):
                                pss = [pp2.tile([128, 512], F32,
                                                name="pqk", tag="pqk")
                                       for _ in range(2)]
                                for kp in range(KC // 2):
                                    for bb in range(2):
                                        b = half * 2 + bb
                                        nc.tensor.matmul(
                                            pss[bb][:, 0:S],
                                            wt[:, 2 * kp:2 * kp + 2, :],
                                            x3[b][:, 2 * kp:2 * kp + 2, 0:S],
                                            start=(kp == 0), stop=(kp == 3),
                                            perf_mode=DR)
                                for bb in range(2):
                                    b = half * 2 + bb
                                    if with_qkbias:
                                        nc.vector.tensor_scalar_add(
                                            out=ot[:, b * S:(b + 1) * S],
                                            in0=pss[bb][:, 0:S],
                                            scalar1=b_sb[:, ec:ec + 1])
                                    else:
                                        nc.vector.tensor_copy(
                                            out=ot[:, b * S:(b + 1) * S],
                                            in_=pss[bb][:, 0:S])
                            qkt.append(ot)
                        qte, kte = qkt

                        # tail key (j=256) for all 4 batches: packed into
                        # array col groups 32b / row groups 64*hi, one
                        # shared per-hi exp over all rows.
                        ps_t = [pp2.tile([128, 512], F32, name="ps_t",
                                         tag="pqk") for _ in range(2)]
                        et_t = [e_p.tile([128, S], BF16, name="et_t",
                                         tag="et") for _ in range(2)]
                        for hi in range(2):
                            p0 = hi * 64
                            for b in range(B_LOC):
                                nc.tensor.matmul(
                                    ps_t[hi][32 * b:32 * b + 1, 0:S],
                                    kte[p0:p0 + 64,
                                        b * S + 256: b * S + 257],
                                    qte[p0:p0 + 64, b * S:(b + 1) * S],
                                    start=True, stop=True,
                                    tile_position=(p0, 32 * b))
                            if with_mask:
                                for b in range(B_LOC):
                                    nc.vector.tensor_add(
                                        out=ps_t[hi][32 * b:32 * b + 1, 0:S],
                                        in0=ps_t[hi][32 * b:32 * b + 1, 0:S],
                                        in1=msk[(b, 2)][0:1, :])
                            nc.scalar.activation(out=et_t[hi][0:97, :],
                                                 in_=ps_t[hi][0:97, 0:S],
                                                 func=AF.Exp, scale=dq_qk)

                        for b in range(B_LOC):
                            ets = []
                            for jc, (j0, jcs) in enumerate(JC[:2]):
                                sp = psp.tile([128, 2, 512], F32,
                                              name="sp", tag="sp")
                                for hi in range(2):
                                    p0 = hi * 64
                                    nc.tensor.matmul(
                                        sp[0:jcs, hi, 0:S],
                                        kte[p0:p0 + 64,
                                            b * S + j0: b * S + j0 + jcs],
                                        qte[p0:p0 + 64,
                                            b * S:(b + 1) * S],
                                        start=True, stop=True)
                                if with_mask:
                                    for hi in range(2):
                                        nc.vector.tensor_add(
                                            out=sp[0:jcs, hi, 0:S],
                                            in0=sp[0:jcs, hi, 0:S],
                                            in1=msk[(b, jc)][0:jcs, :])
                                et = e_p.tile([128, 2, S], BF16,
                                              name="et", tag="et2")
                                nc.scalar.activation(
                                    out=et[0:jcs, :, :],
                                    in_=sp[0:jcs, :, 0:S], func=AF.Exp,
                                    scale=dq_qk)
                                ets.append(et)
                            cp = psp.tile([128, 2, 512], F32,
                                          name="cp", tag="sp")
                            for hi in range(2):
                                h = 2 * ec + hi
                                for jc, (j0, jcs) in enumerate(JC[:2]):
                                    nc.tensor.matmul(
                                        cp[0:128, hi, 0:S],
                                        v_tiles[(b, jc)][0:jcs, h, :],
                                        ets[jc][0:jcs, hi, :],
                                        start=(jc == 0), stop=False)
                                nc.tensor.matmul(
                                    cp[0:128, hi, 0:S],
                                    v_tiles[(b, 2)][32 * b:32 * b + 1, h, :],
                                    et_t[hi][32 * b:32 * b + 1, :],
                                    start=False, stop=True,
                                    tile_position=(32 * b, 0))
                            rst = rs_p.tile([64, 2, S], F32,
                                            name="rst", tag="rst")
                            nc.vector.reciprocal_approx_fast(
                                out=rst[0:64, :, :],
                                in_=cp[0:64, :, 0:S])
                            for hi in range(2):
                                nc.vector.tensor_mul(
                                    out=ctx3[b][hi * 64:hi * 64 + 64,
                                                ec, 0:S],
                                    in0=cp[64:128, hi, 0:S],
                                    in1=rst[0:64, hi, :])

            # ============= out projection + residual =================
            ht_p = top.enter_context(
                tc.tile_pool(name="ht3", bufs=1, side="right"))
            ht3 = ht_p.tile([128, KC, NT], F32, name="ht3", tag="ht3")
            htb_p = top.enter_context(
                tc.tile_pool(name="htb3", bufs=1, side="right"))
            htb3 = htb_p.tile([128, KC, NT], BF16, name="htb3", tag="htb3")
            with ExitStack() as ph:
                wo_p = ph.enter_context(tc.tile_pool(name="wo", bufs=5))
                xt_p = ph.enter_context(
                    tc.tile_pool(name="xt", bufs=3))
                tox_p = ph.enter_context(
                    tc.tile_pool(name="tox", bufs=4))
                ppo = ph.enter_context(
                    tc.tile_pool(name="ppo", bufs=2, space="PSUM"))
                for m in range(MC_E):
                    wt = wo_p.tile([128, KC, 128], FP8, name="wo", tag="wo")
                    nc.sync.dma_start(out=wt[:], in_=ow_d[m, :, :, :])
                    xtm = xt_p.tile([128, NT], F32, name="xt", tag="xt")
                    nc.sync.dma_start(
                        out=xtm[:], in_=xT_d[m * 128:(m + 1) * 128, :])
                    ps = ppo.tile([128, B_LOC, 512], F32, name="po",
                                  tag="po")
                    for kp in range(KC // 2):
                        for b in range(B_LOC):
                            nc.tensor.matmul(
                                ps[:, b, 0:S],
                                wt[:, 2 * kp:2 * kp + 2, :],
                                ctx3[b][:, 2 * kp:2 * kp + 2, 0:S],
                                start=(kp == 0), stop=(kp == 3),
                                perf_mode=DR)
                    for b in range(B_LOC):
                        bs = slice(b * S, (b + 1) * S)
                        if b < 2:
                            nc.vector.scalar_tensor_tensor(
                                out=ht3[:, m, bs], in0=ps[:, b, 0:S],
                                scalar=dq_o, in1=xtm[:, bs],
                                op0=ALU.mult, op1=ALU.add)
                        else:
                            # ACT dequants from PSUM, GpSimd adds residual
                            # (GpSimd cannot read PSUM)
                            to = tox_p.tile([128, S], F32, name="to",
                                            tag="to")
                            nc.scalar.mul(out=to[:], in_=ps[:, b, 0:S],
                                          mul=dq_o)
                            nc.gpsimd.tensor_add(out=ht3[:, m, bs],
                                                 in0=to[:], in1=xtm[:, bs])
                        if with_obias:
                            nc.vector.tensor_scalar_add(
                                out=ht3[:, m, bs], in0=ht3[:, m, bs],
                                scalar1=ob_sb[:, m:m + 1])
                    nc.vector.tensor_copy(out=htb3[:, m, :],
                                          in_=ht3[:, m, :])
        # ctx3 closed

        # ================= LN2 + MLP =====================================
        with tc.tile_pool(name="xln2", bufs=B_LOC) as xln2_p:
            xln2_3 = [xln2_p.tile([128, KC, SP], FP8, tag="x3b", name="x3b")
                      for _ in range(B_LOC)]
            f1o_p = top.enter_context(
                tc.tile_pool(name="f1o", bufs=MC_I, side="right"))
            f1o = []
            with ExitStack() as ln2_ph:
                spin_p2 = ln2_ph.enter_context(
                    tc.tile_pool(name="spin2", bufs=1, space="PSUM"))
                spin_t2 = spin_p2.tile([1, 512], F32, name="spin", tag="spin")
                emit_ln(ln2_ph, htb3, xln2_3, "2", apply3=ht3,
                        spin_t=spin_t2, spin=24)
                wf1_p = ln2_ph.enter_context(tc.tile_pool(name="wf1", bufs=6))
                ppf1 = ln2_ph.enter_context(
                    tc.tile_pool(name="ppf1", bufs=2, space="PSUM"))
                for m in range(MC_I):
                    wt = wf1_p.tile([128, KC, 128], FP8, name="wf1",
                                    tag="wf1")
                    nc.sync.dma_start(out=wt[:], in_=f1w_d[m, :, :, :])
                    o = f1o_p.tile([128, NT], BF16, name="f1o", tag="f1o")
                    for half in range(2):
                        ps = ppf1.tile([128, 2, 512], F32, name="pf1",
                                       tag="pf1")
                        for kp in range(KC // 2):
                            for bb in range(2):
                                b = half * 2 + bb
                                nc.tensor.matmul(
                                    ps[:, bb, 0:S],
                                    wt[:, 2 * kp:2 * kp + 2, :],
                                    xln2_3[b][:, 2 * kp:2 * kp + 2, 0:S],
                                    start=(kp == 0), stop=(kp == 3),
                                    perf_mode=DR)
                        nc.scalar.activation(
                            out=o[:, half * 2 * S:(half + 1) * 2 * S],
                            in_=ps[:, :, 0:S],
                            func=AF.Gelu_apprx_tanh,
                            bias=f1b_sb[:, m:m + 1],
                            scale=dq_f1)
                    f1o.append(o)

        with ExitStack() as ph:
            wf2_p = ph.enter_context(tc.tile_pool(name="wf2", bufs=3))
            ppf2 = ph.enter_context(
                tc.tile_pool(name="ppf2", bufs=2, space="PSUM"))
            out_p = ph.enter_context(tc.tile_pool(name="outp", bufs=3))
            for m in range(MC_E):
                wt = wf2_p.tile([128, MC_I, 128], BF16, name="wf2", tag="wf2")
                nc.sync.dma_start(out=wt[:], in_=f2w_d[m, :, :, :])
                ps = ppf2.tile([128, B_LOC, 512], F32, name="pf2", tag="pf2")
                for b in range(B_LOC):
                    for k in range(MC_I):
                        nc.tensor.matmul(
                            ps[:, b, 0:S], wt[:, k, :],
                            f1o[k][:, b * S:(b + 1) * S],
                            start=(k == 0), stop=(k == MC_I - 1))
                o = out_p.tile([128, NT], F32, name="oo", tag="oo")
                nc.vector.scalar_tensor_tensor(
                    out=o[:], in0=ps[:, :, 0:S], scalar=f2b_sb[:, m:m + 1],
                    in1=ht3[:, m, :], op0=ALU.add, op1=ALU.add)
                nc.sync.dma_start(out=outT_d[m * 128:(m + 1) * 128, :],
                                  in_=o[:])

    nc.compile()
    return nc


FP8_NP = ml_dtypes.float8_e4m3fn


def _q8(W, s):
    """Quantize W*s to e4m3 (clipped to TRN max normal 240)."""
    return np.clip(np.asarray(W, np.float32) * s, -240, 240).astype(FP8_NP)


def _pack_lhsT8(W, s):
    """W [M, K] (out, in) -> [M/128, 128, K/128, 128] fp8 with
    [m, p, k, j] = W[m*128+j, k*128+p]*s (lhsT tiles, partition = K)."""
    W = np.asarray(W, np.float32)
    M, K = W.shape
    A = W.reshape(M // 128, 128, K // 128, 128)
    return _q8(np.ascontiguousarray(A.transpose(0, 3, 2, 1)), s)


def _pack_lhsT(W):
    """bf16 variant of _pack_lhsT8 (no scale)."""
    W = np.asarray(W, np.float32)
    M, K = W.shape
    A = W.reshape(M // 128, 128, K // 128, 128)
    return np.ascontiguousarray(A.transpose(0, 3, 2, 1)).astype(ml_dtypes.bfloat16)


def _pack_pbias(b):
    """b [M] -> [128, M/128] f32 per-partition bias columns."""
    return np.ascontiguousarray(np.asarray(b, np.float32).reshape(-1, 128).T)


def _wscale(W):
    return float(120.0 / max(np.abs(np.asarray(W, np.float32)).max(), 1e-30))


def kernel(hidden_states, attention_mask, causal_attention_mask,
           ln1_w, ln1_b, q_w, q_b, k_w, k_b, v_w, v_b, o_w, o_b,
           ln2_w, ln2_b, fc1_w, fc1_b, fc2_w, fc2_b):
    global LAST_EXEC_NS
    from concourse.bass_utils import run_bass_kernel_spmd

    hs = np.asarray(hidden_states, np.float32)
    msk = (np.asarray(attention_mask, np.float32)
           + np.asarray(causal_attention_mask, np.float32))
    with_mask = bool(np.any(msk))

    ln1_w = np.asarray(ln1_w, np.float32); ln1_b = np.asarray(ln1_b, np.float32)
    ln2_w = np.asarray(ln2_w, np.float32); ln2_b = np.asarray(ln2_b, np.float32)
    q_w = np.asarray(q_w, np.float32); q_b = np.asarray(q_b, np.float32)
    k_w = np.asarray(k_w, np.float32); k_b = np.asarray(k_b, np.float32)
    v_w = np.asarray(v_w, np.float32); v_b = np.asarray(v_b, np.float32)
    o_w = np.asarray(o_w, np.float32); o_b = np.asarray(o_b, np.float32)
    fc1_w = np.asarray(fc1_w, np.float32); fc1_b = np.asarray(fc1_b, np.float32)
    fc2_w = np.asarray(fc2_w, np.float32); fc2_b = np.asarray(fc2_b, np.float32)

    scale = D ** -0.5
    # fold LN1 scale/bias into Q/K/V, and the softmax scale into Q
    qw_eff = (q_w * ln1_w[None, :]) * scale
    qb_eff = (q_b + q_w @ ln1_b) * scale
    kw_eff = k_w * ln1_w[None, :]
    kb_eff = k_b + k_w @ ln1_b
    vw_eff = v_w * ln1_w[None, :]
    vb_eff = v_b + v_w @ ln1_b
    # fold LN2 into fc1
    f1w_eff = fc1_w * ln2_w[None, :]
    f1b_eff = fc1_b + fc1_w @ ln2_b

    # fp8 weight scales (LN activations are pre-scaled by S_X on device)
    s_wq = _wscale(qw_eff)
    s_wk = _wscale(kw_eff)
    s_wv = _wscale(vw_eff)
    s_wo = _wscale(o_w)
    s_wf1 = _wscale(f1w_eff)
    dq_qk = 1.0 / (S_X * S_X * s_wq * s_wk)
    dq_v = 1.0 / (S_X * s_wv)
    dq_o = 1.0 / (S_CTX * s_wo)
    dq_f1 = 1.0 / (S_X * s_wf1)

    # vw: [E_in, E_out] grouped into k-pairs -> [KC/2, 128, 2, E] fp8
    vw_t = np.ascontiguousarray(vw_eff.T.reshape(KC, 128, E))
    vw_pk = np.ascontiguousarray(
        vw_t.reshape(KC // 2, 2, 128, E).transpose(0, 2, 1, 3))

    base = {
        "qw": _pack_lhsT8(qw_eff, s_wq),
        "kw": _pack_lhsT8(kw_eff, s_wk),
        "vw": _q8(vw_pk, s_wv),
        "ow": _pack_lhsT8(o_w, s_wo),
        "f1w": _pack_lhsT8(f1w_eff, s_wf1),
        "f2w": _pack_lhsT(fc2_w),
        "qb": _pack_pbias(qb_eff * (S_X * s_wq)),
        "kb": _pack_pbias(kb_eff * (S_X * s_wk)),
        "vb": np.ascontiguousarray(vb_eff[None, :].astype(np.float32)),
        "ob": _pack_pbias(o_b),
        "f1b": _pack_pbias(f1b_eff),
        "f2b": _pack_pbias(fc2_b),
    }

    with_vbias = bool(np.any(vb_eff))
    with_qkbias = bool(np.any(qb_eff)) or bool(np.any(kb_eff))
    with_obias = bool(np.any(o_b))
    key = (with_mask, with_vbias, with_qkbias, with_obias,
           dq_v, dq_qk, dq_o, dq_f1)
    if key not in _cache:
        _cache[key] = _build(with_mask, with_vbias, with_qkbias, with_obias,
                             dq_v, dq_qk, dq_o, dq_f1)
    nc = _cache[key]

    in_maps = []
    for c in range(N_CORES):
        x = hs[c * B_LOC:(c + 1) * B_LOC].reshape(NT, E).T
        m = dict(base)
        m["xT"] = np.ascontiguousarray(x)
        m["xTb"] = np.ascontiguousarray(x).astype(ml_dtypes.bfloat16)
        if with_mask:
            m["mskT"] = np.ascontiguousarray(
                msk[c * B_LOC:(c + 1) * B_LOC, 0].transpose(0, 2, 1)
                / dq_qk)
        in_maps.append(m)

    res = run_bass_kernel_spmd(nc, in_maps, core_ids=list(range(N_CORES)),
                               trace=TRACE)
    LAST_EXEC_NS = res.exec_time_ns

    outs = []
    for c in range(N_CORES):
        oT = res.results[c]["outT"]          # [E, NT] f32
        outs.append(np.ascontiguousarray(oT.T).reshape(B_LOC, S, E))
    return np.concatenate(outs, axis=0)
